# revision 38
# baseline (speedup 1.0000x reference)
"""Trainium2 Bass kernel for a CapsuleNet dynamic-routing layer.

Math (per batch element b):
    u_hat[b,i,o,d] = sum_k W[i,o,d,k] * x[b,i,k]      # B=256, IC=1152, OC=10, OD=16, ID=8
    b_log = 0
    for it in 0..2:
        c = softmax(b_log, axis=o)
        s[b,o,d] = sum_i c[b,i,o] * u_hat[b,i,o,d]
        v = squash(s)
        if it < 2: b_log += sum_d u_hat * v

Sharding: data-parallel over B across 8 cores (32 local rows), W replicated.

Per-core layout: partition axis p = bp*16 + ii (bp = b%8, ii = i%16); the
u_hat build contracts q = ii*8 + k with a host-prepacked block-diagonal x
(lhsT) against the W stack (rhs), one matmul per (iblk, bblk).  u_hat lives in
SBUF as [128(p), 72(iblk), 4(bblk), 160(o*16+d)] fp16.

Engine assignment / schedule (tuned against the TimelineSim cost model;
modeled duration 213 us vs 272 us for the previous version):
 - c-weighted i-reduction on PE via a c-blockdiagonal lhsT [128, (bp,o)=80]
   built by ONE DVE broadcast-mask multiply per (bblk, iter) -- replaces 64
   SWDGE copies (994 ns fixed cost each on the Pool engine).
 - iter-0 s0 = 0.1*sum_i u_hat as a single 72-matmul chain with M=32 (the
   old version used 288 matmuls; PE cost is N-columns only, Ldweights free).
 - agreement (sum_d u*v) is a fp16 elementwise multiply + in-place
   pairwise-add tree (DVE 2x mode, 0.575 ns/el), with a j-tail offloaded to
   GpSimd (1.98 ns/el).  GpSimd cannot read PSUM, so build drains go to
   ACT/DVE only.
 - the build runs in two bblk-pair halves; the iter-0 agreement for bblks
   0/1 plus the first full routing section are interleaved into the second
   half so DVE/PE never wait on a phase barrier.
 - logits/c tensors are [p, iblk, o] with o innermost so DVE ops hit 2x mode.
"""

import sys

sys.path.insert(0, "/opt/trn_rl_repo")

from contextlib import ExitStack

import numpy as np

import concourse.bass as bass
import concourse.tile as tile
from concourse import mybir

BF = mybir.dt.float16
F32 = mybir.dt.float32
AX = mybir.AxisListType
AF = mybir.ActivationFunctionType

N_CORES = 8
B_FULL, IC, OC, OD, ID = 256, 1152, 10, 16, 8
B_LOC = B_FULL // N_CORES          # 32
NIB = IC // 16                     # 72 i-blocks of 16
NBB = B_LOC // 8                   # 4 b-blocks of 8
F = OC * OD                        # 160

# agreement j-block split across engines (DVE fast, Pool helper); the split
# equalizes modeled total busy: DVE 0.575 ns/el (2x TT) vs Pool 1.98 (mul) +
# 1.39 (reduce)
AGR_SLICES = ((0, 26, "v"), (26, 52, "v"), (52, 62, "p"), (62, 72, "p"))


def _squash(nc, smp, ps, scale, vout, P, groups=1):
    """vout = squash(scale * ps), ps a [P, groups*160] psum view (f32)."""
    G = groups * OC
    sq = smp.tile([P, groups * F], F32, tag="sq")
    nc.scalar.activation(sq[:], ps[:], AF.Square, scale=float(scale))
    n2 = smp.tile([P, G], F32, tag="n2")
    nc.vector.tensor_reduce(
        n2[:], sq[:].rearrange("p (o d) -> p o d", d=OD), axis=AX.X,
        op=mybir.AluOpType.add)
    n1 = smp.tile([P, G], F32, tag="n1")
    nc.scalar.add(n1[:], n2[:], 1.0)
    r1 = smp.tile([P, G], F32, tag="r1")
    nc.vector.reciprocal(r1[:], n1[:])
    sn = smp.tile([P, G], F32, tag="sn")
    nc.scalar.sqrt(sn[:], n2[:])
    sne = smp.tile([P, G], F32, tag="sne")
    nc.vector.tensor_scalar_add(sne[:], sn[:], 1e-8)
    r2 = smp.tile([P, G], F32, tag="r2")
    nc.vector.reciprocal(r2[:], sne[:])
    f1 = smp.tile([P, G], F32, tag="f1")
    nc.vector.tensor_mul(f1[:], n2[:], r1[:])
    nc.vector.tensor_mul(f1[:], f1[:], r2[:])
    if scale != 1.0:
        nc.scalar.mul(f1[:], f1[:], float(scale))
    nc.vector.tensor_mul(
        vout.rearrange("p (o d) -> p o d", d=OD),
        ps.rearrange("p (o d) -> p o d", d=OD),
        f1[:].unsqueeze(-1).broadcast_to((P, G, OD)))


def _split_multiwait(nc):
    """The walrus in this container encodes at most ONE semaphore wait on
    Matmult/Ldweights and HWDGE DMACopy instructions ("Too many sync wait
    commands").  Hoist excess waits onto same-engine NoOps placed directly
    before the instruction - position-identical semantics, ~2 cycles each.
    SWDGE (Pool-queue) DMAs handle multi-waits fine and are left alone.
    """
    for fn in nc.m.functions:
        for bb in fn.blocks:
            out = []
            k = 0
            for ins in bb.instructions:
                si = ins.sync_info
                waits = list(si.on_wait) if si is not None and si.on_wait else []
                limit = 1
                if ins.opcode == "DMACopy":
                    q = str(getattr(ins, "queue", "") or "")
                    if "HW" in q and len(waits) > 1:
                        raise AssertionError(
                            f"HWDGE DMA {ins.name} has {len(waits)} waits: {ins}")
                if len(waits) > limit:
                    for w in waits[:-limit]:
                        nop = mybir.InstNoOp(name=f"{ins.name}-wn{k}", ins=[], outs=[])
                        k += 1
                        nop.engine = ins.engine
                        nop.sync_info = mybir.SyncInfo(on_wait=[w], on_update=[])
                        out.append(nop)
                    ins.sync_info = mybir.SyncInfo(
                        on_wait=waits[-limit:],
                        on_update=list(si.on_update) if si.on_update else [])
                out.append(ins)
            bb.instructions = out


def build_program(split_waits=True):
    """split_waits=True applies the walrus 1-wait workaround (required for
    hardware compiles); CoreSim/TimelineSim need the unsplit program."""
    nc = bass.Bass()
    bd_d = nc.declare_dram_parameter("bd", [2, 8, 128, 9, 2, 128], BF, isOutput=False)
    xt_d = nc.declare_dram_parameter("xt", [128, NIB, B_LOC], BF, isOutput=False)
    ws_d = nc.declare_dram_parameter("ws", [8, 128, 9, F], BF, isOutput=False)
    msk_d = nc.declare_dram_parameter("msk", [80, F], BF, isOutput=False)
    o80_d = nc.declare_dram_parameter("o80", [80, 8], BF, isOutput=False)
    sel_d = nc.declare_dram_parameter("sel", [8, 128], BF, isOutput=False)
    sel32_d = nc.declare_dram_parameter("sel32", [32, NBB, 128], BF, isOutput=False)
    m80_d = nc.declare_dram_parameter("m80", [128, 8, OC], BF, isOutput=False)
    out_d = nc.declare_dram_parameter("out", [B_LOC, F], F32, isOutput=True)

    with ExitStack() as ctx:
        tc = ctx.enter_context(tile.TileContext(nc))
        st = ctx.enter_context(tc.tile_pool(name="st", bufs=1))
        bdp = ctx.enter_context(tc.tile_pool(name="bdp", bufs=2))
        cbp = ctx.enter_context(tc.tile_pool(name="cbp", bufs=2))
        y2p = ctx.enter_context(tc.tile_pool(name="y2p", bufs=2))
        y2g = ctx.enter_context(tc.tile_pool(name="y2g", bufs=2))
        t1p = ctx.enter_context(tc.tile_pool(name="t1p", bufs=2))
        tsp = ctx.enter_context(tc.tile_pool(name="tsp", bufs=2))
        mkp = ctx.enter_context(tc.tile_pool(name="mkp", bufs=2))
        vxp = ctx.enter_context(tc.tile_pool(name="vxp", bufs=2))
        smp = ctx.enter_context(tc.tile_pool(name="smp", bufs=3))
        pbig = ctx.enter_context(tc.tile_pool(name="pbig", bufs=5, space="PSUM"))
        psml = ctx.enter_context(tc.tile_pool(name="psml", bufs=3, space="PSUM"))

        # --- persistent tiles ---
        u_hat = st.tile([128, NIB, NBB, F], BF, tag="u_hat")
        ws_sb = st.tile([128, 8, 9, F], BF, tag="ws_sb")
        blg = st.tile([128, NBB, NIB, OC], BF, tag="blg")
        c_sb = st.tile([128, NBB, NIB, OC], BF, tag="c_sb")
        msk_sb = st.tile([80, F], BF, tag="msk_sb")
        o80_sb = st.tile([80, 8], BF, tag="o80_sb")
        sel_sb = st.tile([8, 128], BF, tag="sel_sb")
        sel32_sb = st.tile([32, NBB, 128], BF, tag="sel32_sb")
        m80_sb = st.tile([128, 8, OC], BF, tag="m80_sb")
        xt_sb = st.tile([128, NIB, B_LOC], BF, tag="xt_sb")
        v32 = st.tile([32, F], BF, tag="v32")
        v8 = [st.tile([8, F], BF, tag=f"v8_{i}", name=f"v8_{i}") for i in range(NBB)]
        vx0t = [st.tile([128, F], BF, tag=f"vx0_{i}", name=f"vx0_{i}")
                for i in range(NBB)]
        of32 = st.tile([8, NBB, F], F32, tag="of32")

        # --- input loads ---
        nc.sync.dma_start(out=xt_sb[:], in_=xt_d[:])
        for e in range(8):
            eng = nc.sync if e % 2 == 0 else nc.scalar
            eng.dma_start(out=ws_sb[:, e], in_=ws_d[e])
        nc.scalar.dma_start(out=sel32_sb[:], in_=sel32_d[:])
        nc.sync.dma_start(out=msk_sb[:], in_=msk_d[:])
        nc.sync.dma_start(out=o80_sb[:], in_=o80_d[:])
        nc.scalar.dma_start(out=sel_sb[:], in_=sel_d[:])
        nc.sync.dma_start(out=m80_sb[:], in_=m80_d[:])

        # --- pass 1: iter-0 uniform-c reduction s0 = 0.1*sum_i u_hat computed
        # directly as x @ W over the full (i,k) contraction: one 72-matmul
        # chain, M=32 (all local b at once). ---
        ps0 = psml.tile([32, F], F32, tag="psml", name="ps0")
        for e in range(8):
            for j in range(9):
                iblk = e * 9 + j
                nc.tensor.matmul(
                    ps0[:], lhsT=xt_sb[:, iblk, :], rhs=ws_sb[:, e, j, :],
                    start=(iblk == 0), stop=(iblk == NIB - 1))

        # --- iter-0 squash + vx replication emitted BEFORE the build so the
        # PE runs them right after pass 1 and the iter-0 agreement (DVE) can
        # overlap the build (its muls self-gate on u_hat subtile drains). ---
        _squash(nc, smp, ps0[:], 0.1, v32[:], 32)
        vx0 = []
        for bblk in range(NBB):
            pvx = psml.tile([128, F], F32, tag="psml", name=f"pvx0_{bblk}")
            nc.tensor.matmul(
                pvx[:], lhsT=sel32_sb[:, bblk, :], rhs=v32[:],
                start=True, stop=True)
            nc.scalar.copy(vx0t[bblk][:], pvx[:])
            vx0.append(vx0t[bblk])

        def agr_slice(bblk, first, vx, j0, j1, eng, pool, cap):
            nj = j1 - j0
            y2 = pool.tile([128, cap, F], BF, tag="y2")
            y2s = y2[:, 0:nj, :]
            eng.tensor_mul(
                y2s, u_hat[:, j0:j1, bblk, :],
                vx[:].unsqueeze(1).broadcast_to((128, nj, F)))
            y2v = y2s.rearrange("p j (o d) -> p j o d", d=OD)
            dst = blg[:, bblk, j0:j1, :]
            # in-place pairwise-add tree over d (fp16, 2x on DVE)
            eng.tensor_add(
                y2v[:, :, :, 0:8], y2v[:, :, :, 0:8], y2v[:, :, :, 8:16])
            eng.tensor_add(
                y2v[:, :, :, 0:4], y2v[:, :, :, 0:4], y2v[:, :, :, 4:8])
            eng.tensor_add(
                y2v[:, :, :, 0:2], y2v[:, :, :, 0:2], y2v[:, :, :, 2:4])
            if first:
                eng.tensor_add(dst, y2v[:, :, :, 0], y2v[:, :, :, 1])
            else:
                ts = tsp.tile([128, 29, OC], BF, tag="ts")
                tss = ts[:, 0:nj, :]
                eng.tensor_add(tss, y2v[:, :, :, 0], y2v[:, :, :, 1])
                eng.tensor_add(dst, dst, tss)

        # --- pass 2: build u_hat in two bblk-pair halves (h=0: bblks 0,1;
        # h=1: bblks 2,3).  After emitting half 0, the iter-0 agreement for
        # bblks 0,1 and iter-1's first softmax+slab are emitted so they run
        # on DVE while the PE builds half 1.  Drains: ACT 2/3, Pool 1/3. ---
        def build_half(h, e_range=range(8)):
            for e in e_range:
                bdt = bdp.tile([128, 9, 2, 128], BF, tag="bdt")
                nc.gpsimd.dma_start(out=bdt[:], in_=bd_d[h, e])
                for j in range(9):
                    iblk = e * 9 + j
                    ps = pbig.tile([128, 2, F], F32, tag="pbig")
                    for bb in range(2):
                        nc.tensor.matmul(
                            ps[:, bb, :], lhsT=bdt[:, j, bb, :],
                            rhs=ws_sb[:, e, j, :], start=True, stop=True)
                    dst = u_hat[:, iblk, h * 2:(h + 1) * 2, :]
                    # h0: DVE is idle, split drains evenly; h1: DVE runs the
                    # iter-0 agreement, ACT takes all drains
                    if h == 1 or iblk % 2 == 0:
                        nc.scalar.copy(dst, ps[:])
                    else:
                        nc.vector.tensor_copy(dst, ps[:])

        build_half(0)
        # b0/b1 iter-0 agreement DVE part (on the critical path to iter 1;
        # overlaps the h=1 build).  The j>=52 tail goes to Pool, emitted
        # after build_half(1) so it does not block the h=1 bd-DMA issues.
        for bblk in (0, 1):
            for (j0, j1) in ((0, 24), (24, 48), (48, 72)):
                agr_slice(bblk, True, vx0[bblk], j0, j1, nc.vector, y2p, 29)

        # --- iters 1, 2: software-pipelined per bblk.  softmax+slab of the
        # NEXT b-block is emitted between stage-1 and the agreement of the
        # current one, keeping DVE busy while the PE runs stage-1. ---
        def softslab(bblk):
            # softmax over o for this b-block (no max-sub: |logits| << 1)
            nc.scalar.activation(c_sb[:, bblk], blg[:, bblk], AF.Exp)
            sm = smp.tile([128, NIB], F32, tag="sm")
            nc.vector.tensor_reduce(
                sm[:], c_sb[:, bblk], axis=AX.X, op=mybir.AluOpType.add)
            rr = smp.tile([128, NIB], F32, tag="rr")
            nc.vector.reciprocal(rr[:], sm[:])
            nc.vector.tensor_mul(
                c_sb[:, bblk], c_sb[:, bblk],
                rr[:].unsqueeze(-1).broadcast_to((128, NIB, OC)))
            # c-blockdiag slab [p, j, (bp', o)] via one broadcast-mask mul
            cbt = cbp.tile([128, NIB, 8, OC], BF, tag="cbt")
            nc.vector.tensor_mul(
                cbt[:],
                c_sb[:, bblk].unsqueeze(2).broadcast_to((128, NIB, 8, OC)),
                m80_sb[:].unsqueeze(1).broadcast_to((128, NIB, 8, OC)))
            return cbt

        state = {"cbt": None}
        sections = [(it, bblk) for it in (1, 2) for bblk in range(NBB)]

        def section(idx, defer_pool=None):
            it, bblk = sections[idx]
            cbt = state["cbt"]
            # stage 1+2: s = diag(C^T U) via blockdiag-c, o-mask, reduce
            ps1 = pbig.tile([80, F], F32, tag="pbig", name=f"ps1_{bblk}")
            for j in range(NIB):
                nc.tensor.matmul(
                    ps1[:], lhsT=cbt[:, j, :, :],
                    rhs=u_hat[:, j, bblk, :],
                    start=(j == 0), stop=(j == NIB - 1))
            mskd = mkp.tile([80, F], BF, tag="mskd")
            nc.vector.tensor_mul(mskd[:], ps1[:], msk_sb[:])
            if it == 1:
                psv = psml.tile([8, F], F32, tag="psml")
                nc.tensor.matmul(
                    psv[:], lhsT=o80_sb[:], rhs=mskd[:],
                    start=True, stop=True)
                _squash(nc, smp, psv[:], 1.0, v8[bblk][:], 8)
            else:
                if bblk % 2 == 0:
                    state["psvp"] = psml.tile([8, 2, F], F32, tag="psml",
                                              name=f"psvp{bblk}")
                nc.tensor.matmul(
                    state["psvp"][:, bblk % 2, :], lhsT=o80_sb[:], rhs=mskd[:],
                    start=True, stop=True)
                if bblk % 2 == 1:
                    # batched squash over a pair of b-blocks
                    g0 = bblk - 1
                    _squash(nc, smp,
                            state["psvp"][:].rearrange("p g f -> p (g f)"),
                            1.0,
                            of32[:, g0:g0 + 2, :].rearrange(
                                "p g f -> p (g f)"), 8, groups=2)
                if bblk == NBB - 1:
                    nc.gpsimd.dma_start(
                        out=out_d[:].rearrange("(g p) f -> p g f", g=NBB),
                        in_=of32[:])
            # emit next section's softmax+slab before this one's agreement
            if idx + 1 < len(sections):
                cbt_next = softslab(sections[idx + 1][1])
            if it == 1:
                pvx = psml.tile([128, F], F32, tag="psml", name=f"pvx1_{bblk}")
                nc.tensor.matmul(
                    pvx[:], lhsT=sel_sb[:], rhs=v8[bblk][:],
                    start=True, stop=True)
                vx = vxp.tile([128, F], BF, tag="vx", name=f"vx1_{bblk}")
                nc.scalar.copy(vx[:], pvx[:])
                for (j0, j1, ekey) in AGR_SLICES:
                    if ekey == "v":
                        agr_slice(bblk, False, vx, j0, j1, nc.vector, y2p, 29)
                    elif defer_pool is not None:
                        defer_pool.append((bblk, vx, j0, j1))
                    else:
                        agr_slice(bblk, False, vx, j0, j1, nc.gpsimd, y2g, 12)
            if idx + 1 < len(sections):
                state["cbt"] = cbt_next

        # iter-1's first softmax+slab emitted before the h=1 build so its ACT
        # exp is queued ahead of the h=1 drains and DVE finishes the slab
        # while the PE is still building
        state["cbt"] = softslab(0)
        build_half(1, range(0, 4))
        # the whole first routing section runs interleaved with the second
        # part of the h=1 build (its Pool work is deferred so the bd-DMA
        # issues are not blocked on the in-order Pool queue)
        deferred = []
        section(0, defer_pool=deferred)
        build_half(1, range(4, 8))
        for (bblk, vx, j0, j1) in deferred:
            agr_slice(bblk, False, vx, j0, j1, nc.gpsimd, y2g, 12)
        # b2/b3 iter-0 agreement split DVE/Pool (consumed only by iter-1's
        # later b-blocks, so it can trail into the routing phase)
        for bblk in (2, 3):
            for (j0, j1, ekey) in AGR_SLICES:
                eng = nc.vector if ekey == "v" else nc.gpsimd
                pool, cap = (y2p, 29) if ekey == "v" else (y2g, 12)
                agr_slice(bblk, True, vx0[bblk], j0, j1, eng, pool, cap)

        for idx in range(1, len(sections)):
            section(idx)

    if split_waits:
        _split_multiwait(nc)
    return nc


def _host_inputs(x, W):
    """Per-core input maps from full x [256,1152,8] f32, W [1,1152,10,16,8] f32."""
    bf = np.float16
    W0 = np.asarray(W[0], dtype=np.float32)
    # ws[e, q=(ii,k), j, (o,d)] = W[(e*9+j)*16+ii, o, d, k]
    ws = np.ascontiguousarray(
        W0.reshape(8, 9, 16, OC, OD, ID).transpose(0, 2, 5, 1, 3, 4)
        .reshape(8, 128, 9, F)).astype(bf)
    msk = np.zeros((80, F), dtype=bf)
    for bpp in range(8):
        for o in range(OC):
            msk[bpp * 10 + o, o * OD:(o + 1) * OD] = 1.0
    o80 = np.zeros((80, 8), dtype=bf)
    for p in range(80):
        o80[p, p // 10] = 1.0
    sel = np.zeros((8, 128), dtype=bf)
    for p in range(128):
        sel[p // 16, p] = 1.0
    sel32 = np.zeros((32, NBB, 128), dtype=bf)
    for bb in range(NBB):
        for p in range(128):
            sel32[bb * 8 + p // 16, bb, p] = 1.0
    m80 = np.zeros((128, 8, OC), dtype=bf)
    for p in range(128):
        m80[p, p // 16, :] = 1.0

    in_maps = []
    for c in range(N_CORES):
        xc = np.asarray(x[c * B_LOC:(c + 1) * B_LOC], dtype=np.float32)
        # bd[e, q=(ii,k), j, bb, m=(bp,ii')] = x[bb*8+bp, (e*9+j)*16+ii, k] iff ii'==ii
        r = xc.reshape(NBB, 8, 8, 9, 16, ID)          # [bb, bp, e, j, ii, k]
        bd6 = np.zeros((8, 16, ID, 9, NBB, 8, 16), dtype=np.float32)
        for ii in range(16):
            # [bb, bp, e, j, k] -> [e, k, j, bb, bp]
            bd6[:, ii, :, :, :, :, ii] = r[:, :, :, :, ii, :].transpose(2, 4, 3, 0, 1)
        # [e, q=(ii,k), j, bb, m] -> [h, e, q, j, bb%2, m]
        bd = np.ascontiguousarray(
            bd6.reshape(8, 128, 9, 2, 2, 128).transpose(3, 0, 1, 2, 4, 5)).astype(bf)
        # xt[q=(ii,k), iblk, b] = x[b, iblk*16+ii, k]
        xt = np.ascontiguousarray(
            xc.reshape(B_LOC, NIB, 16, ID).transpose(2, 3, 1, 0)
            .reshape(128, NIB, B_LOC)).astype(bf)
        in_maps.append(
            {"bd": bd, "xt": xt, "ws": ws, "msk": msk, "o80": o80,
             "sel": sel, "sel32": sel32, "m80": m80})
    return in_maps


_NC_CACHE = {}


def kernel(x, W):
    from concourse.bass_utils import run_bass_kernel_spmd

    if "nc" not in _NC_CACHE:
        _NC_CACHE["nc"] = build_program()
    nc = _NC_CACHE["nc"]
    in_maps = _host_inputs(x, W)
    res = run_bass_kernel_spmd(nc, in_maps, core_ids=list(range(N_CORES)))
    out = np.concatenate([r["out"] for r in res.results], axis=0)
    return out.reshape(B_FULL, OC, OD).astype(np.float32)


if __name__ == "__main__":
    nc = build_program()
    print("program built ok")


# revision 45
# speedup vs baseline: 1.0334x; 1.0334x over previous
"""Trainium2 Bass kernel for a CapsuleNet dynamic-routing layer.

Math (per batch element b):
    u_hat[b,i,o,d] = sum_k W[i,o,d,k] * x[b,i,k]      # B=256, IC=1152, OC=10, OD=16, ID=8
    b_log = 0
    for it in 0..2:
        c = softmax(b_log, axis=o)
        s[b,o,d] = sum_i c[b,i,o] * u_hat[b,i,o,d]
        v = squash(s)
        if it < 2: b_log += sum_d u_hat * v

Sharding: data-parallel over B across 8 cores (32 local rows), W replicated.

Per-core layout: partition axis p = bp*16 + ii (bp = b%8, ii = i%16); the
u_hat build contracts q = ii*8 + k with a host-prepacked block-diagonal x
(lhsT) against the W stack (rhs), one matmul per (iblk, bblk).  u_hat lives in
SBUF as [128(p), 72(iblk), 4(bblk), 160(o*16+d)] fp16.

Engine assignment / schedule (tuned against the TimelineSim cost model;
modeled duration 213 us vs 272 us for the previous version):
 - c-weighted i-reduction on PE via a c-blockdiagonal lhsT [128, (bp,o)=80]
   built by ONE DVE broadcast-mask multiply per (bblk, iter) -- replaces 64
   SWDGE copies (994 ns fixed cost each on the Pool engine).
 - iter-0 s0 = 0.1*sum_i u_hat as a single 72-matmul chain with M=32 (the
   old version used 288 matmuls; PE cost is N-columns only, Ldweights free).
 - agreement (sum_d u*v) is a fp16 elementwise multiply + in-place
   pairwise-add tree (DVE 2x mode, 0.575 ns/el), with a j-tail offloaded to
   GpSimd (1.98 ns/el).  GpSimd cannot read PSUM, so build drains go to
   ACT/DVE only.
 - the build runs in two bblk-pair halves; the iter-0 agreement for bblks
   0/1 plus the first full routing section are interleaved into the second
   half so DVE/PE never wait on a phase barrier.
 - logits/c tensors are [p, iblk, o] with o innermost so DVE ops hit 2x mode.
"""

import sys

sys.path.insert(0, "/opt/trn_rl_repo")

from contextlib import ExitStack

import numpy as np

import concourse.bass as bass
import concourse.tile as tile
from concourse import mybir

BF = mybir.dt.float16
F32 = mybir.dt.float32
AX = mybir.AxisListType
AF = mybir.ActivationFunctionType

N_CORES = 8
B_FULL, IC, OC, OD, ID = 256, 1152, 10, 16, 8
B_LOC = B_FULL // N_CORES          # 32
NIB = IC // 16                     # 72 i-blocks of 16
NBB = B_LOC // 8                   # 4 b-blocks of 8
F = OC * OD                        # 160

# agreement j-block split across engines (DVE fast, Pool helper); the split
# equalizes modeled total busy: DVE 0.575 ns/el (2x TT) vs Pool 1.98 (mul) +
# 1.39 (reduce)
AGR_SLICES = ((0, 29, "v"), (29, 57, "v"), (57, 65, "p"), (65, 72, "p"))


def _squash(nc, smp, ps, scale, vout, P, groups=1):
    """vout = squash(scale * ps), ps a [P, groups*160] psum view (f32)."""
    G = groups * OC
    sq = smp.tile([P, groups * F], F32, tag="sq")
    nc.scalar.activation(sq[:], ps[:], AF.Square, scale=float(scale))
    n2 = smp.tile([P, G], F32, tag="n2")
    nc.vector.tensor_reduce(
        n2[:], sq[:].rearrange("p (o d) -> p o d", d=OD), axis=AX.X,
        op=mybir.AluOpType.add)
    n1 = smp.tile([P, G], F32, tag="n1")
    nc.scalar.add(n1[:], n2[:], 1.0)
    r1 = smp.tile([P, G], F32, tag="r1")
    nc.vector.reciprocal(r1[:], n1[:])
    sn = smp.tile([P, G], F32, tag="sn")
    nc.scalar.sqrt(sn[:], n2[:])
    sne = smp.tile([P, G], F32, tag="sne")
    nc.vector.tensor_scalar_add(sne[:], sn[:], 1e-8)
    r2 = smp.tile([P, G], F32, tag="r2")
    nc.vector.reciprocal(r2[:], sne[:])
    f1 = smp.tile([P, G], F32, tag="f1")
    nc.vector.tensor_mul(f1[:], n2[:], r1[:])
    nc.vector.tensor_mul(f1[:], f1[:], r2[:])
    if scale != 1.0:
        nc.scalar.mul(f1[:], f1[:], float(scale))
    nc.vector.tensor_mul(
        vout.rearrange("p (o d) -> p o d", d=OD),
        ps.rearrange("p (o d) -> p o d", d=OD),
        f1[:].unsqueeze(-1).broadcast_to((P, G, OD)))


def _split_multiwait(nc):
    """The walrus in this container encodes at most ONE semaphore wait on
    Matmult/Ldweights and HWDGE DMACopy instructions ("Too many sync wait
    commands").  Hoist excess waits onto same-engine NoOps placed directly
    before the instruction - position-identical semantics, ~2 cycles each.
    SWDGE (Pool-queue) DMAs handle multi-waits fine and are left alone.
    """
    for fn in nc.m.functions:
        for bb in fn.blocks:
            out = []
            k = 0
            for ins in bb.instructions:
                si = ins.sync_info
                waits = list(si.on_wait) if si is not None and si.on_wait else []
                limit = 1
                if ins.opcode == "DMACopy":
                    q = str(getattr(ins, "queue", "") or "")
                    if "HW" in q and len(waits) > 1:
                        raise AssertionError(
                            f"HWDGE DMA {ins.name} has {len(waits)} waits: {ins}")
                if len(waits) > limit:
                    for w in waits[:-limit]:
                        nop = mybir.InstNoOp(name=f"{ins.name}-wn{k}", ins=[], outs=[])
                        k += 1
                        nop.engine = ins.engine
                        nop.sync_info = mybir.SyncInfo(on_wait=[w], on_update=[])
                        out.append(nop)
                    ins.sync_info = mybir.SyncInfo(
                        on_wait=waits[-limit:],
                        on_update=list(si.on_update) if si.on_update else [])
                out.append(ins)
            bb.instructions = out


def build_program(split_waits=True):
    """split_waits=True applies the walrus 1-wait workaround (required for
    hardware compiles); CoreSim/TimelineSim need the unsplit program."""
    nc = bass.Bass()
    bd_d = nc.declare_dram_parameter("bd", [2, 8, 128, 9, 2, 128], BF, isOutput=False)
    xt_d = nc.declare_dram_parameter("xt", [128, NIB, B_LOC], BF, isOutput=False)
    ws_d = nc.declare_dram_parameter("ws", [8, 128, 9, F], BF, isOutput=False)
    msk_d = nc.declare_dram_parameter("msk", [80, F], BF, isOutput=False)
    o80_d = nc.declare_dram_parameter("o80", [80, 8], BF, isOutput=False)
    sel_d = nc.declare_dram_parameter("sel", [8, 128], BF, isOutput=False)
    sel32_d = nc.declare_dram_parameter("sel32", [32, NBB, 128], BF, isOutput=False)
    m80_d = nc.declare_dram_parameter("m80", [128, 8, OC], BF, isOutput=False)
    out_d = nc.declare_dram_parameter("out", [B_LOC, F], F32, isOutput=True)

    with ExitStack() as ctx:
        tc = ctx.enter_context(tile.TileContext(nc))
        st = ctx.enter_context(tc.tile_pool(name="st", bufs=1))
        bdp = ctx.enter_context(tc.tile_pool(name="bdp", bufs=2))
        cbp = ctx.enter_context(tc.tile_pool(name="cbp", bufs=2))
        y2p = ctx.enter_context(tc.tile_pool(name="y2p", bufs=2))
        y2g = ctx.enter_context(tc.tile_pool(name="y2g", bufs=2))
        t1p = ctx.enter_context(tc.tile_pool(name="t1p", bufs=2))
        tsp = ctx.enter_context(tc.tile_pool(name="tsp", bufs=2))
        mkp = ctx.enter_context(tc.tile_pool(name="mkp", bufs=2))
        vxp = ctx.enter_context(tc.tile_pool(name="vxp", bufs=2))
        smp = ctx.enter_context(tc.tile_pool(name="smp", bufs=3))
        pbig = ctx.enter_context(tc.tile_pool(name="pbig", bufs=5, space="PSUM"))
        psml = ctx.enter_context(tc.tile_pool(name="psml", bufs=3, space="PSUM"))

        # --- persistent tiles ---
        u_hat = st.tile([128, NIB, NBB, F], BF, tag="u_hat")
        ws_sb = st.tile([128, 8, 9, F], BF, tag="ws_sb")
        blg = st.tile([128, NBB, NIB, OC], BF, tag="blg")
        c_sb = st.tile([128, NBB, NIB, OC], BF, tag="c_sb")
        msk_sb = st.tile([80, F], BF, tag="msk_sb")
        o80_sb = st.tile([80, 8], BF, tag="o80_sb")
        sel_sb = st.tile([8, 128], BF, tag="sel_sb")
        sel32_sb = st.tile([32, NBB, 128], BF, tag="sel32_sb")
        m80_sb = st.tile([128, 8, OC], BF, tag="m80_sb")
        xt_sb = st.tile([128, NIB, B_LOC], BF, tag="xt_sb")
        v32 = st.tile([32, F], BF, tag="v32")
        v8 = [st.tile([8, F], BF, tag=f"v8_{i}", name=f"v8_{i}") for i in range(NBB)]
        vx0t = [st.tile([128, F], BF, tag=f"vx0_{i}", name=f"vx0_{i}")
                for i in range(NBB)]
        of32 = st.tile([8, NBB, F], F32, tag="of32")

        # --- input loads ---
        nc.sync.dma_start(out=xt_sb[:], in_=xt_d[:])
        for e in range(8):
            eng = nc.sync if e % 2 == 0 else nc.scalar
            eng.dma_start(out=ws_sb[:, e], in_=ws_d[e])
        nc.scalar.dma_start(out=sel32_sb[:], in_=sel32_d[:])
        nc.sync.dma_start(out=msk_sb[:], in_=msk_d[:])
        nc.sync.dma_start(out=o80_sb[:], in_=o80_d[:])
        nc.scalar.dma_start(out=sel_sb[:], in_=sel_d[:])
        nc.sync.dma_start(out=m80_sb[:], in_=m80_d[:])

        # --- pass 1: iter-0 uniform-c reduction s0 = 0.1*sum_i u_hat computed
        # directly as x @ W over the full (i,k) contraction: one 72-matmul
        # chain, M=32 (all local b at once). ---
        ps0 = psml.tile([32, F], F32, tag="psml", name="ps0")
        for e in range(8):
            for j in range(9):
                iblk = e * 9 + j
                nc.tensor.matmul(
                    ps0[:], lhsT=xt_sb[:, iblk, :], rhs=ws_sb[:, e, j, :],
                    start=(iblk == 0), stop=(iblk == NIB - 1))

        # --- iter-0 squash + vx replication emitted BEFORE the build so the
        # PE runs them right after pass 1 and the iter-0 agreement (DVE) can
        # overlap the build (its muls self-gate on u_hat subtile drains). ---
        _squash(nc, smp, ps0[:], 0.1, v32[:], 32)
        vx0 = []
        for bblk in range(NBB):
            pvx = psml.tile([128, F], F32, tag="psml", name=f"pvx0_{bblk}")
            nc.tensor.matmul(
                pvx[:], lhsT=sel32_sb[:, bblk, :], rhs=v32[:],
                start=True, stop=True)
            nc.scalar.copy(vx0t[bblk][:], pvx[:])
            vx0.append(vx0t[bblk])

        def agr_slice(bblk, first, vx, j0, j1, eng, pool, cap):
            nj = j1 - j0
            y2 = pool.tile([128, cap, F], BF, tag="y2")
            y2s = y2[:, 0:nj, :]
            eng.tensor_mul(
                y2s, u_hat[:, j0:j1, bblk, :],
                vx[:].unsqueeze(1).broadcast_to((128, nj, F)))
            y2v = y2s.rearrange("p j (o d) -> p j o d", d=OD)
            dst = blg[:, bblk, j0:j1, :]
            # in-place pairwise-add tree over d (fp16, 2x on DVE)
            eng.tensor_add(
                y2v[:, :, :, 0:8], y2v[:, :, :, 0:8], y2v[:, :, :, 8:16])
            eng.tensor_add(
                y2v[:, :, :, 0:4], y2v[:, :, :, 0:4], y2v[:, :, :, 4:8])
            eng.tensor_add(
                y2v[:, :, :, 0:2], y2v[:, :, :, 0:2], y2v[:, :, :, 2:4])
            if first:
                eng.tensor_add(dst, y2v[:, :, :, 0], y2v[:, :, :, 1])
            else:
                ts = tsp.tile([128, 29, OC], BF, tag="ts")
                tss = ts[:, 0:nj, :]
                eng.tensor_add(tss, y2v[:, :, :, 0], y2v[:, :, :, 1])
                eng.tensor_add(dst, dst, tss)

        # --- pass 2: build u_hat in two bblk-pair halves (h=0: bblks 0,1;
        # h=1: bblks 2,3).  After emitting half 0, the iter-0 agreement for
        # bblks 0,1 and iter-1's first softmax+slab are emitted so they run
        # on DVE while the PE builds half 1.  Drains: ACT 2/3, Pool 1/3. ---
        def build_half(h, e_range=range(8)):
            for e in e_range:
                bdt = bdp.tile([128, 9, 2, 128], BF, tag="bdt")
                nc.gpsimd.dma_start(out=bdt[:], in_=bd_d[h, e])
                for j in range(9):
                    iblk = e * 9 + j
                    ps = pbig.tile([128, 2, F], F32, tag="pbig")
                    for bb in range(2):
                        nc.tensor.matmul(
                            ps[:, bb, :], lhsT=bdt[:, j, bb, :],
                            rhs=ws_sb[:, e, j, :], start=True, stop=True)
                    dst = u_hat[:, iblk, h * 2:(h + 1) * 2, :]
                    # h0: DVE is idle, split drains evenly; h1: DVE runs the
                    # iter-0 agreement, ACT takes all drains
                    if h == 1 or iblk % 2 == 0:
                        nc.scalar.copy(dst, ps[:])
                    else:
                        nc.vector.tensor_copy(dst, ps[:])

        build_half(0)
        # b0/b1 iter-0 agreement DVE part (on the critical path to iter 1;
        # overlaps the h=1 build).  The j>=52 tail goes to Pool, emitted
        # after build_half(1) so it does not block the h=1 bd-DMA issues.
        for bblk in (0, 1):
            for (j0, j1) in ((0, 24), (24, 48), (48, 72)):
                agr_slice(bblk, True, vx0[bblk], j0, j1, nc.vector, y2p, 29)

        # --- iters 1, 2: software-pipelined per bblk.  softmax+slab of the
        # NEXT b-block is emitted between stage-1 and the agreement of the
        # current one, keeping DVE busy while the PE runs stage-1. ---
        def softslab(bblk):
            # softmax over o for this b-block (no max-sub: |logits| << 1)
            nc.scalar.activation(c_sb[:, bblk], blg[:, bblk], AF.Exp)
            sm = smp.tile([128, NIB], F32, tag="sm")
            nc.vector.tensor_reduce(
                sm[:], c_sb[:, bblk], axis=AX.X, op=mybir.AluOpType.add)
            rr = smp.tile([128, NIB], F32, tag="rr")
            nc.vector.reciprocal(rr[:], sm[:])
            nc.vector.tensor_mul(
                c_sb[:, bblk], c_sb[:, bblk],
                rr[:].unsqueeze(-1).broadcast_to((128, NIB, OC)))
            # c-blockdiag slab [p, j, (bp', o)] via one broadcast-mask mul
            cbt = cbp.tile([128, NIB, 8, OC], BF, tag="cbt")
            nc.vector.tensor_mul(
                cbt[:],
                c_sb[:, bblk].unsqueeze(2).broadcast_to((128, NIB, 8, OC)),
                m80_sb[:].unsqueeze(1).broadcast_to((128, NIB, 8, OC)))
            return cbt

        state = {"cbt": None}
        sections = [(it, bblk) for it in (1, 2) for bblk in range(NBB)]

        def section(idx, defer_pool=None):
            it, bblk = sections[idx]
            cbt = state["cbt"]
            # stage 1+2: s = diag(C^T U) via blockdiag-c, o-mask, reduce
            ps1 = pbig.tile([80, F], F32, tag="pbig", name=f"ps1_{bblk}")
            for j in range(NIB):
                nc.tensor.matmul(
                    ps1[:], lhsT=cbt[:, j, :, :],
                    rhs=u_hat[:, j, bblk, :],
                    start=(j == 0), stop=(j == NIB - 1))
            mskd = mkp.tile([80, F], BF, tag="mskd")
            nc.vector.tensor_mul(mskd[:], ps1[:], msk_sb[:])
            if it == 1:
                psv = psml.tile([8, F], F32, tag="psml")
                nc.tensor.matmul(
                    psv[:], lhsT=o80_sb[:], rhs=mskd[:],
                    start=True, stop=True)
                _squash(nc, smp, psv[:], 1.0, v8[bblk][:], 8)
            else:
                if bblk % 2 == 0:
                    state["psvp"] = psml.tile([8, 2, F], F32, tag="psml",
                                              name=f"psvp{bblk}")
                nc.tensor.matmul(
                    state["psvp"][:, bblk % 2, :], lhsT=o80_sb[:], rhs=mskd[:],
                    start=True, stop=True)
                if bblk % 2 == 1:
                    # batched squash over a pair of b-blocks
                    g0 = bblk - 1
                    _squash(nc, smp,
                            state["psvp"][:].rearrange("p g f -> p (g f)"),
                            1.0,
                            of32[:, g0:g0 + 2, :].rearrange(
                                "p g f -> p (g f)"), 8, groups=2)
                if bblk == NBB - 1:
                    nc.gpsimd.dma_start(
                        out=out_d[:].rearrange("(g p) f -> p g f", g=NBB),
                        in_=of32[:])
            # emit next section's softmax+slab before this one's agreement
            if idx + 1 < len(sections):
                cbt_next = softslab(sections[idx + 1][1])
            if it == 1:
                pvx = psml.tile([128, F], F32, tag="psml", name=f"pvx1_{bblk}")
                nc.tensor.matmul(
                    pvx[:], lhsT=sel_sb[:], rhs=v8[bblk][:],
                    start=True, stop=True)
                vx = vxp.tile([128, F], BF, tag="vx", name=f"vx1_{bblk}")
                nc.scalar.copy(vx[:], pvx[:])
                for (j0, j1, ekey) in AGR_SLICES:
                    if ekey == "v":
                        agr_slice(bblk, False, vx, j0, j1, nc.vector, y2p, 29)
                    elif defer_pool is not None:
                        defer_pool.append((bblk, vx, j0, j1))
                    else:
                        agr_slice(bblk, False, vx, j0, j1, nc.gpsimd, y2g, 13)
            if idx + 1 < len(sections):
                state["cbt"] = cbt_next

        # iter-1's first softmax+slab emitted before the h=1 build so its ACT
        # exp is queued ahead of the h=1 drains and DVE finishes the slab
        # while the PE is still building
        state["cbt"] = softslab(0)
        build_half(1, range(0, 4))
        # the whole first routing section runs interleaved with the second
        # part of the h=1 build (its Pool work is deferred so the bd-DMA
        # issues are not blocked on the in-order Pool queue)
        deferred = []
        section(0, defer_pool=deferred)
        build_half(1, range(4, 8))
        for (bblk, vx, j0, j1) in deferred:
            agr_slice(bblk, False, vx, j0, j1, nc.gpsimd, y2g, 13)
        # b2/b3 iter-0 agreement all-DVE: its consumers (softslab of iter-1
        # b2/b3) have tight deadlines and a Pool share serializes behind the
        # deferred agr1 Pool work
        for bblk in (2, 3):
            for (j0, j1) in ((0, 24), (24, 48), (48, 72)):
                agr_slice(bblk, True, vx0[bblk], j0, j1, nc.vector, y2p, 29)

        for idx in range(1, len(sections)):
            section(idx)

    if split_waits:
        _split_multiwait(nc)
    return nc


def _host_inputs(x, W):
    """Per-core input maps from full x [256,1152,8] f32, W [1,1152,10,16,8] f32."""
    bf = np.float16
    W0 = np.asarray(W[0], dtype=np.float32)
    # ws[e, q=(ii,k), j, (o,d)] = W[(e*9+j)*16+ii, o, d, k]
    ws = np.ascontiguousarray(
        W0.reshape(8, 9, 16, OC, OD, ID).transpose(0, 2, 5, 1, 3, 4)
        .reshape(8, 128, 9, F)).astype(bf)
    msk = np.zeros((80, F), dtype=bf)
    for bpp in range(8):
        for o in range(OC):
            msk[bpp * 10 + o, o * OD:(o + 1) * OD] = 1.0
    o80 = np.zeros((80, 8), dtype=bf)
    for p in range(80):
        o80[p, p // 10] = 1.0
    sel = np.zeros((8, 128), dtype=bf)
    for p in range(128):
        sel[p // 16, p] = 1.0
    sel32 = np.zeros((32, NBB, 128), dtype=bf)
    for bb in range(NBB):
        for p in range(128):
            sel32[bb * 8 + p // 16, bb, p] = 1.0
    m80 = np.zeros((128, 8, OC), dtype=bf)
    for p in range(128):
        m80[p, p // 16, :] = 1.0

    in_maps = []
    for c in range(N_CORES):
        xc = np.asarray(x[c * B_LOC:(c + 1) * B_LOC], dtype=np.float32)
        # bd[e, q=(ii,k), j, bb, m=(bp,ii')] = x[bb*8+bp, (e*9+j)*16+ii, k] iff ii'==ii
        r = xc.reshape(NBB, 8, 8, 9, 16, ID)          # [bb, bp, e, j, ii, k]
        bd6 = np.zeros((8, 16, ID, 9, NBB, 8, 16), dtype=np.float32)
        for ii in range(16):
            # [bb, bp, e, j, k] -> [e, k, j, bb, bp]
            bd6[:, ii, :, :, :, :, ii] = r[:, :, :, :, ii, :].transpose(2, 4, 3, 0, 1)
        # [e, q=(ii,k), j, bb, m] -> [h, e, q, j, bb%2, m]
        bd = np.ascontiguousarray(
            bd6.reshape(8, 128, 9, 2, 2, 128).transpose(3, 0, 1, 2, 4, 5)).astype(bf)
        # xt[q=(ii,k), iblk, b] = x[b, iblk*16+ii, k]
        xt = np.ascontiguousarray(
            xc.reshape(B_LOC, NIB, 16, ID).transpose(2, 3, 1, 0)
            .reshape(128, NIB, B_LOC)).astype(bf)
        in_maps.append(
            {"bd": bd, "xt": xt, "ws": ws, "msk": msk, "o80": o80,
             "sel": sel, "sel32": sel32, "m80": m80})
    return in_maps


_NC_CACHE = {}


def kernel(x, W):
    from concourse.bass_utils import run_bass_kernel_spmd

    if "nc" not in _NC_CACHE:
        _NC_CACHE["nc"] = build_program()
    nc = _NC_CACHE["nc"]
    in_maps = _host_inputs(x, W)
    res = run_bass_kernel_spmd(nc, in_maps, core_ids=list(range(N_CORES)))
    out = np.concatenate([r["out"] for r in res.results], axis=0)
    return out.reshape(B_FULL, OC, OD).astype(np.float32)


if __name__ == "__main__":
    nc = build_program()
    print("program built ok")


# revision 49
# speedup vs baseline: 1.0337x; 1.0003x over previous
"""Trainium2 Bass kernel for a CapsuleNet dynamic-routing layer.

Math (per batch element b):
    u_hat[b,i,o,d] = sum_k W[i,o,d,k] * x[b,i,k]      # B=256, IC=1152, OC=10, OD=16, ID=8
    b_log = 0
    for it in 0..2:
        c = softmax(b_log, axis=o)
        s[b,o,d] = sum_i c[b,i,o] * u_hat[b,i,o,d]
        v = squash(s)
        if it < 2: b_log += sum_d u_hat * v

Sharding: data-parallel over B across 8 cores (32 local rows), W replicated.

Per-core layout: partition axis p = bp*16 + ii (bp = b%8, ii = i%16); the
u_hat build contracts q = ii*8 + k with a host-prepacked block-diagonal x
(lhsT) against the W stack (rhs), one matmul per (iblk, bblk).  u_hat lives in
SBUF as [128(p), 72(iblk), 4(bblk), 160(o*16+d)] fp16.

Engine assignment / schedule (tuned against the TimelineSim cost model;
modeled duration 207 us vs 272 us for the previous version):
 - c-weighted i-reduction on PE via a c-blockdiagonal lhsT [128, (bp,o)=80]
   built by ONE DVE broadcast-mask multiply per (bblk, iter) -- replaces 64
   SWDGE copies (994 ns fixed cost each on the Pool engine).
 - iter-0 s0 = 0.1*sum_i u_hat as a single 72-matmul chain with M=32 (the
   old version used 288 matmuls; PE cost is N-columns only, Ldweights free).
 - agreement (sum_d u*v) is a fp16 elementwise multiply + in-place
   pairwise-add tree (DVE 2x mode, 0.575 ns/el); iter-1's j>=57 tail runs on
   GpSimd (1.98 ns/el) whose deadline (next iteration's softmax) is loose.
   Iter-0 agreements are all-DVE: their consumers are on the critical path
   and a Pool share serializes behind other Pool work.  GpSimd cannot read
   PSUM, so build drains go to ACT/DVE only.
 - the build runs in two bblk-pair halves; the iter-0 agreement for bblks
   0/1 plus the first full routing section are interleaved into the second
   half so DVE/PE never wait on a phase barrier.
 - logits/c tensors are [p, iblk, o] with o innermost so DVE ops hit 2x mode.
"""

import sys

sys.path.insert(0, "/opt/trn_rl_repo")

from contextlib import ExitStack

import numpy as np

import concourse.bass as bass
import concourse.tile as tile
from concourse import mybir

BF = mybir.dt.float16
F32 = mybir.dt.float32
AX = mybir.AxisListType
AF = mybir.ActivationFunctionType

N_CORES = 8
B_FULL, IC, OC, OD, ID = 256, 1152, 10, 16, 8
B_LOC = B_FULL // N_CORES          # 32
NIB = IC // 16                     # 72 i-blocks of 16
NBB = B_LOC // 8                   # 4 b-blocks of 8
F = OC * OD                        # 160

# agreement j-block split across engines (DVE fast, Pool helper); the split
# equalizes modeled total busy: DVE 0.575 ns/el (2x TT) vs Pool 1.98 (mul) +
# 1.39 (reduce)
AGR_SLICES = ((0, 29, "v"), (29, 57, "v"), (57, 65, "p"), (65, 72, "p"))


def _squash(nc, smp, ps, scale, vout, P, groups=1):
    """vout = squash(scale * ps), ps a [P, groups*160] psum view (f32)."""
    G = groups * OC
    sq = smp.tile([P, groups * F], F32, tag="sq")
    nc.scalar.activation(sq[:], ps[:], AF.Square, scale=float(scale))
    n2 = smp.tile([P, G], F32, tag="n2")
    nc.vector.tensor_reduce(
        n2[:], sq[:].rearrange("p (o d) -> p o d", d=OD), axis=AX.X,
        op=mybir.AluOpType.add)
    n1 = smp.tile([P, G], F32, tag="n1")
    nc.scalar.add(n1[:], n2[:], 1.0)
    r1 = smp.tile([P, G], F32, tag="r1")
    nc.vector.reciprocal(r1[:], n1[:])
    sn = smp.tile([P, G], F32, tag="sn")
    nc.scalar.sqrt(sn[:], n2[:])
    sne = smp.tile([P, G], F32, tag="sne")
    nc.vector.tensor_scalar_add(sne[:], sn[:], 1e-8)
    r2 = smp.tile([P, G], F32, tag="r2")
    nc.vector.reciprocal(r2[:], sne[:])
    f1 = smp.tile([P, G], F32, tag="f1")
    nc.vector.tensor_mul(f1[:], n2[:], r1[:])
    nc.vector.tensor_mul(f1[:], f1[:], r2[:])
    if scale != 1.0:
        nc.scalar.mul(f1[:], f1[:], float(scale))
    nc.vector.tensor_mul(
        vout.rearrange("p (o d) -> p o d", d=OD),
        ps.rearrange("p (o d) -> p o d", d=OD),
        f1[:].unsqueeze(-1).broadcast_to((P, G, OD)))


def _split_multiwait(nc):
    """The walrus in this container encodes at most ONE semaphore wait on
    Matmult/Ldweights and HWDGE DMACopy instructions ("Too many sync wait
    commands").  Hoist excess waits onto same-engine NoOps placed directly
    before the instruction - position-identical semantics, ~2 cycles each.
    SWDGE (Pool-queue) DMAs handle multi-waits fine and are left alone.
    """
    for fn in nc.m.functions:
        for bb in fn.blocks:
            out = []
            k = 0
            for ins in bb.instructions:
                si = ins.sync_info
                waits = list(si.on_wait) if si is not None and si.on_wait else []
                limit = 1
                if ins.opcode == "DMACopy":
                    q = str(getattr(ins, "queue", "") or "")
                    if "HW" in q and len(waits) > 1:
                        raise AssertionError(
                            f"HWDGE DMA {ins.name} has {len(waits)} waits: {ins}")
                if len(waits) > limit:
                    for w in waits[:-limit]:
                        nop = mybir.InstNoOp(name=f"{ins.name}-wn{k}", ins=[], outs=[])
                        k += 1
                        nop.engine = ins.engine
                        nop.sync_info = mybir.SyncInfo(on_wait=[w], on_update=[])
                        out.append(nop)
                    ins.sync_info = mybir.SyncInfo(
                        on_wait=waits[-limit:],
                        on_update=list(si.on_update) if si.on_update else [])
                out.append(ins)
            bb.instructions = out


def build_program(split_waits=True):
    """split_waits=True applies the walrus 1-wait workaround (required for
    hardware compiles); CoreSim/TimelineSim need the unsplit program."""
    nc = bass.Bass()
    bd_d = nc.declare_dram_parameter("bd", [2, 8, 128, 9, 2, 128], BF, isOutput=False)
    xt_d = nc.declare_dram_parameter("xt", [128, NIB, B_LOC], BF, isOutput=False)
    ws_d = nc.declare_dram_parameter("ws", [8, 128, 9, F], BF, isOutput=False)
    msk_d = nc.declare_dram_parameter("msk", [80, F], BF, isOutput=False)
    o80_d = nc.declare_dram_parameter("o80", [80, 8], BF, isOutput=False)
    sel_d = nc.declare_dram_parameter("sel", [8, 128], BF, isOutput=False)
    sel32_d = nc.declare_dram_parameter("sel32", [32, NBB, 128], BF, isOutput=False)
    m80_d = nc.declare_dram_parameter("m80", [128, 8, OC], BF, isOutput=False)
    out_d = nc.declare_dram_parameter("out", [B_LOC, F], F32, isOutput=True)

    with ExitStack() as ctx:
        tc = ctx.enter_context(tile.TileContext(nc))
        st = ctx.enter_context(tc.tile_pool(name="st", bufs=1))
        bdp = ctx.enter_context(tc.tile_pool(name="bdp", bufs=2))
        cbp = ctx.enter_context(tc.tile_pool(name="cbp", bufs=2))
        y2p = ctx.enter_context(tc.tile_pool(name="y2p", bufs=2))
        y2g = ctx.enter_context(tc.tile_pool(name="y2g", bufs=2))
        t1p = ctx.enter_context(tc.tile_pool(name="t1p", bufs=2))
        tsp = ctx.enter_context(tc.tile_pool(name="tsp", bufs=2))
        mkp = ctx.enter_context(tc.tile_pool(name="mkp", bufs=2))
        vxp = ctx.enter_context(tc.tile_pool(name="vxp", bufs=2))
        smp = ctx.enter_context(tc.tile_pool(name="smp", bufs=3))
        pbig = ctx.enter_context(tc.tile_pool(name="pbig", bufs=5, space="PSUM"))
        psml = ctx.enter_context(tc.tile_pool(name="psml", bufs=3, space="PSUM"))

        # --- persistent tiles ---
        u_hat = st.tile([128, NIB, NBB, F], BF, tag="u_hat")
        ws_sb = st.tile([128, 8, 9, F], BF, tag="ws_sb")
        blg = st.tile([128, NBB, NIB, OC], BF, tag="blg")
        c_sb = st.tile([128, NBB, NIB, OC], BF, tag="c_sb")
        msk_sb = st.tile([80, F], BF, tag="msk_sb")
        o80_sb = st.tile([80, 8], BF, tag="o80_sb")
        sel_sb = st.tile([8, 128], BF, tag="sel_sb")
        sel32_sb = st.tile([32, NBB, 128], BF, tag="sel32_sb")
        m80_sb = st.tile([128, 8, OC], BF, tag="m80_sb")
        xt_sb = st.tile([128, NIB, B_LOC], BF, tag="xt_sb")
        v32 = st.tile([32, F], BF, tag="v32")
        v8 = [st.tile([8, F], BF, tag=f"v8_{i}", name=f"v8_{i}") for i in range(NBB)]
        vx0t = [st.tile([128, F], BF, tag=f"vx0_{i}", name=f"vx0_{i}")
                for i in range(NBB)]
        of32 = st.tile([8, NBB, F], F32, tag="of32")

        # --- input loads ---
        nc.sync.dma_start(out=xt_sb[:], in_=xt_d[:])
        for e in range(8):
            eng = nc.sync if e % 2 == 0 else nc.scalar
            eng.dma_start(out=ws_sb[:, e], in_=ws_d[e])
        nc.scalar.dma_start(out=sel32_sb[:], in_=sel32_d[:])
        nc.sync.dma_start(out=msk_sb[:], in_=msk_d[:])
        nc.sync.dma_start(out=o80_sb[:], in_=o80_d[:])
        nc.scalar.dma_start(out=sel_sb[:], in_=sel_d[:])
        nc.sync.dma_start(out=m80_sb[:], in_=m80_d[:])

        # --- pass 1: iter-0 uniform-c reduction s0 = 0.1*sum_i u_hat computed
        # directly as x @ W over the full (i,k) contraction: one 72-matmul
        # chain, M=32 (all local b at once). ---
        ps0 = psml.tile([32, F], F32, tag="psml", name="ps0")
        for e in range(8):
            for j in range(9):
                iblk = e * 9 + j
                nc.tensor.matmul(
                    ps0[:], lhsT=xt_sb[:, iblk, :], rhs=ws_sb[:, e, j, :],
                    start=(iblk == 0), stop=(iblk == NIB - 1))

        # --- iter-0 squash + vx replication emitted BEFORE the build so the
        # PE runs them right after pass 1 and the iter-0 agreement (DVE) can
        # overlap the build (its muls self-gate on u_hat subtile drains). ---
        _squash(nc, smp, ps0[:], 0.1, v32[:], 32)
        vx0 = []
        for bblk in range(NBB):
            pvx = psml.tile([128, F], F32, tag="psml", name=f"pvx0_{bblk}")
            nc.tensor.matmul(
                pvx[:], lhsT=sel32_sb[:, bblk, :], rhs=v32[:],
                start=True, stop=True)
            nc.scalar.copy(vx0t[bblk][:], pvx[:])
            vx0.append(vx0t[bblk])

        def agr_slice(bblk, first, vx, j0, j1, eng, pool, cap):
            nj = j1 - j0
            y2 = pool.tile([128, cap, F], BF, tag="y2")
            y2s = y2[:, 0:nj, :]
            eng.tensor_mul(
                y2s, u_hat[:, j0:j1, bblk, :],
                vx[:].unsqueeze(1).broadcast_to((128, nj, F)))
            y2v = y2s.rearrange("p j (o d) -> p j o d", d=OD)
            dst = blg[:, bblk, j0:j1, :]
            # in-place pairwise-add tree over d (fp16, 2x on DVE)
            eng.tensor_add(
                y2v[:, :, :, 0:8], y2v[:, :, :, 0:8], y2v[:, :, :, 8:16])
            eng.tensor_add(
                y2v[:, :, :, 0:4], y2v[:, :, :, 0:4], y2v[:, :, :, 4:8])
            eng.tensor_add(
                y2v[:, :, :, 0:2], y2v[:, :, :, 0:2], y2v[:, :, :, 2:4])
            if first:
                eng.tensor_add(dst, y2v[:, :, :, 0], y2v[:, :, :, 1])
            else:
                ts = tsp.tile([128, 29, OC], BF, tag="ts")
                tss = ts[:, 0:nj, :]
                eng.tensor_add(tss, y2v[:, :, :, 0], y2v[:, :, :, 1])
                eng.tensor_add(dst, dst, tss)

        # --- pass 2: build u_hat in two bblk-pair halves (h=0: bblks 0,1;
        # h=1: bblks 2,3).  After emitting half 0, the iter-0 agreement for
        # bblks 0,1 and iter-1's first softmax+slab are emitted so they run
        # on DVE while the PE builds half 1.  Drains: ACT 2/3, Pool 1/3. ---
        def build_half(h, e_range=range(8)):
            for e in e_range:
                bdt = bdp.tile([128, 9, 2, 128], BF, tag="bdt")
                nc.gpsimd.dma_start(out=bdt[:], in_=bd_d[h, e])
                for j in range(9):
                    iblk = e * 9 + j
                    ps = pbig.tile([128, 2, F], F32, tag="pbig")
                    for bb in range(2):
                        nc.tensor.matmul(
                            ps[:, bb, :], lhsT=bdt[:, j, bb, :],
                            rhs=ws_sb[:, e, j, :], start=True, stop=True)
                    dst = u_hat[:, iblk, h * 2:(h + 1) * 2, :]
                    # h0: DVE is idle, split drains evenly; h1: DVE runs the
                    # iter-0 agreement, ACT takes all drains
                    if h == 1 or iblk % 2 == 0:
                        nc.scalar.copy(dst, ps[:])
                    else:
                        nc.vector.tensor_copy(dst, ps[:])

        build_half(0)
        # b0/b1 iter-0 agreement DVE part (on the critical path to iter 1;
        # overlaps the h=1 build).  The j>=52 tail goes to Pool, emitted
        # after build_half(1) so it does not block the h=1 bd-DMA issues.
        for bblk in (0, 1):
            for (j0, j1) in ((0, 24), (24, 48), (48, 72)):
                agr_slice(bblk, True, vx0[bblk], j0, j1, nc.vector, y2p, 29)

        # --- iters 1, 2: software-pipelined per bblk.  softmax+slab of the
        # NEXT b-block is emitted between stage-1 and the agreement of the
        # current one, keeping DVE busy while the PE runs stage-1. ---
        def softslab(bblk):
            # softmax over o for this b-block (no max-sub: |logits| << 1)
            nc.scalar.activation(c_sb[:, bblk], blg[:, bblk], AF.Exp)
            sm = smp.tile([128, NIB], F32, tag="sm")
            nc.vector.tensor_reduce(
                sm[:], c_sb[:, bblk], axis=AX.X, op=mybir.AluOpType.add)
            rr = smp.tile([128, NIB], F32, tag="rr")
            nc.vector.reciprocal(rr[:], sm[:])
            nc.vector.tensor_mul(
                c_sb[:, bblk], c_sb[:, bblk],
                rr[:].unsqueeze(-1).broadcast_to((128, NIB, OC)))
            # c-blockdiag slab [p, j, (bp', o)] via one broadcast-mask mul
            cbt = cbp.tile([128, NIB, 8, OC], BF, tag="cbt")
            nc.vector.tensor_mul(
                cbt[:],
                c_sb[:, bblk].unsqueeze(2).broadcast_to((128, NIB, 8, OC)),
                m80_sb[:].unsqueeze(1).broadcast_to((128, NIB, 8, OC)))
            return cbt

        state = {"cbt": None}
        sections = [(1, 0), (1, 1), (1, 2), (2, 0), (1, 3), (2, 1), (2, 2),
                    (2, 3)]

        def section(idx, defer_pool=None):
            it, bblk = sections[idx]
            cbt = state["cbt"]
            # stage 1+2: s = diag(C^T U) via blockdiag-c, o-mask, reduce
            ps1 = pbig.tile([80, F], F32, tag="pbig", name=f"ps1_{bblk}")
            for j in range(NIB):
                nc.tensor.matmul(
                    ps1[:], lhsT=cbt[:, j, :, :],
                    rhs=u_hat[:, j, bblk, :],
                    start=(j == 0), stop=(j == NIB - 1))
            mskd = mkp.tile([80, F], BF, tag="mskd")
            nc.vector.tensor_mul(mskd[:], ps1[:], msk_sb[:])
            if it == 1:
                psv = psml.tile([8, F], F32, tag="psml")
                nc.tensor.matmul(
                    psv[:], lhsT=o80_sb[:], rhs=mskd[:],
                    start=True, stop=True)
                _squash(nc, smp, psv[:], 1.0, v8[bblk][:], 8)
            else:
                if bblk % 2 == 0:
                    state["psvp"] = psml.tile([8, 2, F], F32, tag="psml",
                                              name=f"psvp{bblk}")
                nc.tensor.matmul(
                    state["psvp"][:, bblk % 2, :], lhsT=o80_sb[:], rhs=mskd[:],
                    start=True, stop=True)
                if bblk % 2 == 1:
                    # batched squash over a pair of b-blocks
                    g0 = bblk - 1
                    _squash(nc, smp,
                            state["psvp"][:].rearrange("p g f -> p (g f)"),
                            1.0,
                            of32[:, g0:g0 + 2, :].rearrange(
                                "p g f -> p (g f)"), 8, groups=2)
                if bblk == NBB - 1:
                    nc.gpsimd.dma_start(
                        out=out_d[:].rearrange("(g p) f -> p g f", g=NBB),
                        in_=of32[:])
            # emit next section's softmax+slab before this one's agreement
            if idx + 1 < len(sections):
                cbt_next = softslab(sections[idx + 1][1])
            if it == 1:
                pvx = psml.tile([128, F], F32, tag="psml", name=f"pvx1_{bblk}")
                nc.tensor.matmul(
                    pvx[:], lhsT=sel_sb[:], rhs=v8[bblk][:],
                    start=True, stop=True)
                vx = vxp.tile([128, F], BF, tag="vx", name=f"vx1_{bblk}")
                nc.scalar.copy(vx[:], pvx[:])
                for (j0, j1, ekey) in AGR_SLICES:
                    if ekey == "v":
                        agr_slice(bblk, False, vx, j0, j1, nc.vector, y2p, 29)
                    elif defer_pool is not None:
                        defer_pool.append((bblk, vx, j0, j1))
                    else:
                        agr_slice(bblk, False, vx, j0, j1, nc.gpsimd, y2g, 13)
            if idx + 1 < len(sections):
                state["cbt"] = cbt_next

        # iter-1's first softmax+slab emitted before the h=1 build so its ACT
        # exp is queued ahead of the h=1 drains and DVE finishes the slab
        # while the PE is still building
        state["cbt"] = softslab(0)
        build_half(1, range(0, 4))
        # the whole first routing section runs interleaved with the second
        # part of the h=1 build (its Pool work is deferred so the bd-DMA
        # issues are not blocked on the in-order Pool queue)
        deferred = []
        section(0, defer_pool=deferred)
        build_half(1, range(4, 8))
        for (bblk, vx, j0, j1) in deferred:
            agr_slice(bblk, False, vx, j0, j1, nc.gpsimd, y2g, 13)
        # b2/b3 iter-0 agreement all-DVE: its consumers (softslab of iter-1
        # b2/b3) have tight deadlines and a Pool share serializes behind the
        # deferred agr1 Pool work
        for bblk in (2, 3):
            for (j0, j1) in ((0, 24), (24, 48), (48, 72)):
                agr_slice(bblk, True, vx0[bblk], j0, j1, nc.vector, y2p, 29)

        for idx in range(1, len(sections)):
            section(idx)

    if split_waits:
        _split_multiwait(nc)
    return nc


def _host_inputs(x, W):
    """Per-core input maps from full x [256,1152,8] f32, W [1,1152,10,16,8] f32."""
    bf = np.float16
    W0 = np.asarray(W[0], dtype=np.float32)
    # ws[e, q=(ii,k), j, (o,d)] = W[(e*9+j)*16+ii, o, d, k]
    ws = np.ascontiguousarray(
        W0.reshape(8, 9, 16, OC, OD, ID).transpose(0, 2, 5, 1, 3, 4)
        .reshape(8, 128, 9, F)).astype(bf)
    msk = np.zeros((80, F), dtype=bf)
    for bpp in range(8):
        for o in range(OC):
            msk[bpp * 10 + o, o * OD:(o + 1) * OD] = 1.0
    o80 = np.zeros((80, 8), dtype=bf)
    for p in range(80):
        o80[p, p // 10] = 1.0
    sel = np.zeros((8, 128), dtype=bf)
    for p in range(128):
        sel[p // 16, p] = 1.0
    sel32 = np.zeros((32, NBB, 128), dtype=bf)
    for bb in range(NBB):
        for p in range(128):
            sel32[bb * 8 + p // 16, bb, p] = 1.0
    m80 = np.zeros((128, 8, OC), dtype=bf)
    for p in range(128):
        m80[p, p // 16, :] = 1.0

    in_maps = []
    for c in range(N_CORES):
        xc = np.asarray(x[c * B_LOC:(c + 1) * B_LOC], dtype=np.float32)
        # bd[e, q=(ii,k), j, bb, m=(bp,ii')] = x[bb*8+bp, (e*9+j)*16+ii, k] iff ii'==ii
        r = xc.reshape(NBB, 8, 8, 9, 16, ID)          # [bb, bp, e, j, ii, k]
        bd6 = np.zeros((8, 16, ID, 9, NBB, 8, 16), dtype=np.float32)
        for ii in range(16):
            # [bb, bp, e, j, k] -> [e, k, j, bb, bp]
            bd6[:, ii, :, :, :, :, ii] = r[:, :, :, :, ii, :].transpose(2, 4, 3, 0, 1)
        # [e, q=(ii,k), j, bb, m] -> [h, e, q, j, bb%2, m]
        bd = np.ascontiguousarray(
            bd6.reshape(8, 128, 9, 2, 2, 128).transpose(3, 0, 1, 2, 4, 5)).astype(bf)
        # xt[q=(ii,k), iblk, b] = x[b, iblk*16+ii, k]
        xt = np.ascontiguousarray(
            xc.reshape(B_LOC, NIB, 16, ID).transpose(2, 3, 1, 0)
            .reshape(128, NIB, B_LOC)).astype(bf)
        in_maps.append(
            {"bd": bd, "xt": xt, "ws": ws, "msk": msk, "o80": o80,
             "sel": sel, "sel32": sel32, "m80": m80})
    return in_maps


_NC_CACHE = {}


def kernel(x, W):
    from concourse.bass_utils import run_bass_kernel_spmd

    if "nc" not in _NC_CACHE:
        _NC_CACHE["nc"] = build_program()
    nc = _NC_CACHE["nc"]
    in_maps = _host_inputs(x, W)
    res = run_bass_kernel_spmd(nc, in_maps, core_ids=list(range(N_CORES)))
    out = np.concatenate([r["out"] for r in res.results], axis=0)
    return out.reshape(B_FULL, OC, OD).astype(np.float32)


if __name__ == "__main__":
    nc = build_program()
    print("program built ok")
